# revision 16
# baseline (speedup 1.0000x reference)
"""ChannelCovarianceBlock Trainium2 kernel.

Computes, for queries x1 (B, C, h, w) and support sets x2 (nw, Bs, C, h, w):
  cov_n = Cov(x2[n].reshape(Bs*C, hw))            (hw, hw) per class
  d     = normalize-and-center rows of x1.reshape(B*C, hw)
  sim[b, n, c] = d[bc] @ cov_n @ d[bc]^T          -> (B, nw*C)

Sharding: data-parallel over B across 8 NeuronCores (32 queries each);
each core computes all 10 class covariances from the full x2 (redundant
but collective-free) using the Gram identity cov = (X^T X - s s^T/N)/(N-1).

Per-core dataflow:
  stage 0: preprocess queries in place (SBUF-resident D), build D^T via
           PE transposes, spill D^T to DRAM.
  stage 1 (per class): Gram matmuls + rank-1 mean correction -> cov_n SBUF.
  stage 2 (per class, m-tile): S = D @ cov_n on PE (float32r), then a
           fused multiply+row-reduce (tensor_tensor_reduce) against the
           resident D gives sim[:, n].
"""

import os
import sys

for _p in ("/opt/trn_rl_repo", "/root/.axon_site/_ro/trn_rl_repo"):
    if os.path.isdir(_p) and _p not in sys.path:
        sys.path.append(_p)

import numpy as np

# Problem constants (hardcoded per spec).
B, C, H, W = 256, 128, 28, 28
NW, BS = 10, 10
HW = H * W            # 784
N_CORES = 8
BSH = B // N_CORES    # 32 queries per core
NI = BSH * C          # 4096 rows per core
NR = BS * C           # 1280 support rows per class

# K-tiles over the hw contraction dim (partition dim <= 128).
KT = [(k * 128, min(128, HW - k * 128)) for k in range((HW + 127) // 128)]
NKT = len(KT)         # 7
# N-tiles over the hw free dim (>=256 keeps float32r at 1 cycle/row).
QT = [(0, 392), (392, 392)]
MT = NI // 128        # 32 i-tiles per core

_STATE = {}


def _build_program(mm_dtype_name="float32r", stages=None):
    if stages is None:
        stages = os.environ.get("CCB_STAGES", "full")
    import concourse.bass as bass
    import concourse.bacc as bacc
    import concourse.tile as tile
    from concourse import mybir
    from concourse.masks import make_identity
    from contextlib import ExitStack

    f32 = mybir.dt.float32
    # Matmul operand dtype: float32r runs the PE at 1 cycle/row (vs 4 for
    # fp32) for N>=256. All f32r-consumed tiles must be f32r-typed with
    # walrus-approved producers (DMA from f32r DRAM, or DVE/ACT rounding
    # copies) -- the BIR verifier enforces this.
    mmdt = getattr(mybir.dt, mm_dtype_name)

    nc = bacc.Bacc()
    x1s = nc.declare_dram_parameter("x1s", [NI, HW], f32, isOutput=False)
    x2d = nc.declare_dram_parameter("x2", [NW, NR, HW], mmdt, isOutput=False)
    out = nc.declare_dram_parameter("out", [NI, NW], f32, isOutput=True)

    AF = mybir.ActivationFunctionType
    OP = mybir.AluOpType

    with tile.TileContext(nc) as tc:
        with ExitStack() as ctx:
            persist = ctx.enter_context(tc.tile_pool(name="persist", bufs=1))
            ident = persist.tile([128, 128], f32, tag="ident")
            make_identity(nc, ident)
            ones_f = persist.tile([128, 1], f32, tag="ones_f")
            nc.vector.memset(ones_f, 1.0)
            ones = persist.tile([128, 1], mmdt, tag="ones")
            nc.vector.tensor_copy(out=ones, in_=ones_f)
            # D stays resident: d_res[:, m, q] = D[m*128 + p, q]
            d_res = persist.tile([128, MT, HW], f32, tag="d_res")
            out_acc = persist.tile([128, MT, NW], f32, tag="out_acc")
            if stages != "full":
                nc.vector.memset(out_acc, 0.0)

            dram = ctx.enter_context(tc.tile_pool(name="dram", bufs=1, space="DRAM"))
            # dtT_dram[m][p, kt, i] = D[m*128 + i, kt*128 + p] (full k-blocks)
            dtT_dram = dram.tile([MT, 128, NKT - 1, 128], mmdt, tag="dtT")
            # remainder k-block (16 rows of p)
            dtr_dram = dram.tile([MT, KT[-1][1], 128], mmdt, tag="dtr")

            scr_pool = ctx.enter_context(tc.tile_pool(name="scr", bufs=2))
            stats = ctx.enter_context(tc.tile_pool(name="stats", bufs=4))

            # ---- Stage 0: query preprocessing + D^T build ----
            with tc.tile_pool(name="psum_t", bufs=2, space="PSUM") as psum_t, \
                 tc.tile_pool(name="dtw", bufs=2) as dtw_pool:
                for m in range(MT):
                    rows = slice(m * 128, (m + 1) * 128)
                    dsl = d_res[:, m, :]
                    nc.sync.dma_start(out=dsl, in_=x1s[rows, :])
                    sq = scr_pool.tile([128, HW], f32, tag="scr")
                    sumsq = stats.tile([128, 1], f32, tag="sumsq")
                    # ACT: sq = x^2 (discarded), sumsq = row-sum(x^2)
                    nc.scalar.activation(
                        out=sq, in_=dsl, func=AF.Square, accum_out=sumsq
                    )
                    s1 = stats.tile([128, 1], f32, tag="s1")
                    nc.vector.tensor_reduce(
                        out=s1, in_=dsl, axis=mybir.AxisListType.X, op=OP.add
                    )
                    nrm = stats.tile([128, 1], f32, tag="nrm")
                    nc.scalar.activation(out=nrm, in_=sumsq, func=AF.Sqrt)
                    rn = stats.tile([128, 1], f32, tag="rn")
                    nc.vector.reciprocal(out=rn, in_=nrm)
                    ms = stats.tile([128, 1], f32, tag="ms")
                    nc.scalar.mul(out=ms, in_=s1, mul=1.0 / HW)
                    # d = (x - mean) * (1/||x||), in place
                    nc.vector.tensor_scalar(
                        out=dsl, in0=dsl, scalar1=ms, scalar2=rn,
                        op0=OP.subtract, op1=OP.mult,
                    )
                    dtw = dtw_pool.tile([128, NKT - 1, 128], mmdt, tag="dtw")
                    dtr = dtw_pool.tile([KT[-1][1], 128], mmdt, tag="dtr")
                    for kt, (koff, klen) in enumerate(KT):
                        pt = psum_t.tile([128, 128], f32, tag="pt")
                        nc.tensor.transpose(
                            out=pt[:klen, :128],
                            in_=dsl[:, koff:koff + klen],
                            identity=ident,
                        )
                        dst = dtw[:, kt, :] if kt < NKT - 1 else dtr
                        nc.vector.tensor_copy(out=dst, in_=pt[:klen, :128])
                    nc.sync.dma_start(out=dtT_dram[m], in_=dtw)
                    nc.sync.dma_start(out=dtr_dram[m], in_=dtr)

            # ---- Stages 1+2 per class ----
            xs_pool = ctx.enter_context(tc.tile_pool(name="xsup", bufs=1))
            cov_pool = ctx.enter_context(tc.tile_pool(name="cov", bufs=2))
            row_pool = ctx.enter_context(tc.tile_pool(name="rows", bufs=1))
            dts_pool = ctx.enter_context(tc.tile_pool(name="dts", bufs=3))
            psum_s = ctx.enter_context(
                tc.tile_pool(name="psum_s", bufs=4, space="PSUM")
            )
            psum_m = ctx.enter_context(
                tc.tile_pool(name="psum_mean", bufs=1, space="PSUM")
            )

            RTN = NR // 128  # 10 row-tiles per class
            for n in range(NW if stages != "0" else 0):
                xs = xs_pool.tile([128, RTN, HW], mmdt, tag="xs")
                for rt in range(RTN):
                    nc.sync.dma_start(
                        out=xs[:, rt, :], in_=x2d[n, rt * 128:(rt + 1) * 128, :]
                    )
                # column sums s (1, HW) via ones-matmul; psum sub-tiles are
                # bank-aligned (512-elem stride) so no matmul crosses a bank.
                pm = psum_m.tile([1, len(QT), 512], f32, tag="pm")
                for rt in range(RTN):
                    for qi, (qoff, qlen) in enumerate(QT):
                        nc.tensor.matmul(
                            pm[:1, qi, :qlen],
                            lhsT=ones[:, :1],
                            rhs=xs[:, rt, qoff:qoff + qlen],
                            start=(rt == 0),
                            stop=(rt == RTN - 1),
                        )
                srow = row_pool.tile([1, HW], mmdt, tag="srow")
                ssrow = row_pool.tile([1, HW], mmdt, tag="ssrow")
                for qi, (qoff, qlen) in enumerate(QT):
                    qs = slice(qoff, qoff + qlen)
                    nc.scalar.mul(out=srow[:, qs], in_=pm[:1, qi, :qlen], mul=1.0)
                    nc.scalar.mul(out=ssrow[:, qs], in_=pm[:1, qi, :qlen], mul=-1.0 / NR)

                # cov_n = (X^T X - s s^T / NR) / (NR - 1), tiled (p-block, q)
                cov = cov_pool.tile([128, NKT, HW], mmdt, tag="cov")
                for mc, (mcoff, mclen) in enumerate(KT):
                    for (qoff, qlen) in QT:
                        ps = psum_s.tile([128, 392], f32, tag="ps")
                        for rt in range(RTN):
                            nc.tensor.matmul(
                                ps[:mclen, :qlen],
                                lhsT=xs[:, rt, mcoff:mcoff + mclen],
                                rhs=xs[:, rt, qoff:qoff + qlen],
                                start=(rt == 0),
                                stop=False,
                            )
                        nc.tensor.matmul(
                            ps[:mclen, :qlen],
                            lhsT=ssrow[:1, mcoff:mcoff + mclen],
                            rhs=srow[:1, qoff:qoff + qlen],
                            start=False,
                            stop=True,
                        )
                        nc.scalar.mul(
                            out=cov[:mclen, mc, qoff:qoff + qlen],
                            in_=ps[:mclen, :qlen],
                            mul=1.0 / (NR - 1),
                        )

                # Stage 2: sim[:, n] = rowsum((D @ cov_n) * D) per m-tile
                for m in range(MT if stages not in ("0", "01") else 0):
                    dtw2 = dts_pool.tile([128, NKT - 1, 128], mmdt, tag="dts")
                    nc.sync.dma_start(out=dtw2, in_=dtT_dram[m])
                    dtr2 = dts_pool.tile([KT[-1][1], 128], mmdt, tag="dtsr")
                    nc.sync.dma_start(out=dtr2, in_=dtr_dram[m])
                    if stages == "2d":
                        scr = scr_pool.tile([128, HW], f32, tag="scr")
                        nc.vector.tensor_copy(out=scr[:, :128], in_=dtw2[:, 0, :])
                        continue
                    acc = out_acc[:, m, n:n + 1]
                    pp = stats.tile([128, 2], f32, name="pp", tag="pp")
                    for qi, (qoff, qlen) in enumerate(QT):
                        ps = psum_s.tile([128, 392], f32, tag="ps")
                        for kt, (koff, klen) in enumerate(KT):
                            lhsT = dtw2[:, kt, :] if kt < NKT - 1 else dtr2
                            nc.tensor.matmul(
                                ps[:128, :qlen],
                                lhsT=lhsT,
                                rhs=cov[:klen, kt, qoff:qoff + qlen],
                                start=(kt == 0),
                                stop=(kt == NKT - 1),
                            )
                        scr = scr_pool.tile([128, HW], f32, tag="scr")
                        # out=(ps*1)*d elementwise; accum_out = row-sum
                        nc.vector.scalar_tensor_tensor(
                            out=scr[:, :qlen],
                            in0=ps[:, :qlen],
                            scalar=1.0,
                            in1=d_res[:, m, qoff:qoff + qlen],
                            op0=OP.mult,
                            op1=OP.mult,
                            accum_out=pp[:, qi:qi + 1],
                        )
                    nc.vector.tensor_reduce(
                        out=acc, in_=pp, axis=mybir.AxisListType.X, op=OP.add
                    )

            for m in range(MT):
                nc.sync.dma_start(
                    out=out[m * 128:(m + 1) * 128, :], in_=out_acc[:, m, :]
                )

    # Bacc defers register allocation to compile(); run_bass_via_pjrt
    # serializes the module as-is, so finalize here.
    nc.finalize()
    return nc


def get_program():
    key = "nc"
    if key not in _STATE:
        _STATE[key] = _build_program(
            os.environ.get("CCB_MM_DTYPE", "float32r")
        )
    return _STATE[key]


def make_in_maps(x1, x2):
    x1f = np.ascontiguousarray(
        np.asarray(x1, dtype=np.float32).reshape(B * C, HW)
    )
    x2f = np.ascontiguousarray(
        np.asarray(x2, dtype=np.float32).reshape(NW, NR, HW)
    )
    return [
        {"x1s": x1f[c * NI:(c + 1) * NI], "x2": x2f}
        for c in range(N_CORES)
    ]


def assemble_output(core_outs):
    # per-core (NI, NW) -> (BSH, NW*C); concat over cores -> (B, NW*C)
    parts = [
        o.reshape(BSH, C, NW).transpose(0, 2, 1).reshape(BSH, NW * C)
        for o in core_outs
    ]
    return np.ascontiguousarray(np.concatenate(parts, axis=0), dtype=np.float32)


def kernel(x1, x2):
    from concourse.bass_utils import run_bass_kernel_spmd

    nc = get_program()
    in_maps = make_in_maps(x1, x2)
    res = run_bass_kernel_spmd(nc, in_maps, list(range(N_CORES)))
    return assemble_output([res.results[i]["out"] for i in range(N_CORES)])


# revision 19
# speedup vs baseline: 1.0424x; 1.0424x over previous
"""ChannelCovarianceBlock Trainium2 kernel.

Computes, for queries x1 (B, C, h, w) and support sets x2 (nw, Bs, C, h, w):
  cov_n = Cov(x2[n].reshape(Bs*C, hw))            (hw, hw) per class
  d     = normalize-and-center rows of x1.reshape(B*C, hw)
  sim[b, n, c] = d[bc] @ cov_n @ d[bc]^T          -> (B, nw*C)

Sharding: data-parallel over B across 8 NeuronCores (32 queries each);
each core computes all 10 class covariances from the full x2 (redundant
but collective-free) using the Gram identity cov = (X^T X - s s^T/N)/(N-1).

Per-core dataflow:
  stage 0: preprocess queries in place (SBUF-resident D), build D^T via
           PE transposes, spill D^T to DRAM.
  stage 1 (per class): Gram matmuls + rank-1 mean correction -> cov_n SBUF.
  stage 2 (per class, m-tile): S = D @ cov_n on PE (float32r), then a
           fused multiply+row-reduce (tensor_tensor_reduce) against the
           resident D gives sim[:, n].
"""

import os
import sys

for _p in ("/opt/trn_rl_repo", "/root/.axon_site/_ro/trn_rl_repo"):
    if os.path.isdir(_p) and _p not in sys.path:
        sys.path.append(_p)

import numpy as np

# Problem constants (hardcoded per spec).
B, C, H, W = 256, 128, 28, 28
NW, BS = 10, 10
HW = H * W            # 784
N_CORES = 8
BSH = B // N_CORES    # 32 queries per core
NI = BSH * C          # 4096 rows per core
NR = BS * C           # 1280 support rows per class

# K-tiles over the hw contraction dim (partition dim <= 128).
KT = [(k * 128, min(128, HW - k * 128)) for k in range((HW + 127) // 128)]
NKT = len(KT)         # 7
# N-tiles over the hw free dim (>=256 keeps float32r at 1 cycle/row).
QT = [(0, 392), (392, 392)]
MT = NI // 128        # 32 i-tiles per core

_STATE = {}


def _build_program(mm_dtype_name="float32r", stages=None, repeat=None):
    if stages is None:
        stages = os.environ.get("CCB_STAGES", "full")
    if repeat is None:
        repeat = int(os.environ.get("CCB_REPEAT", "1"))
    import concourse.bass as bass
    import concourse.bacc as bacc
    import concourse.tile as tile
    from concourse import mybir
    from concourse.masks import make_identity
    from contextlib import ExitStack

    f32 = mybir.dt.float32
    # Matmul operand dtype: float32r runs the PE at 1 cycle/row (vs 4 for
    # fp32) for N>=256. All f32r-consumed tiles must be f32r-typed with
    # walrus-approved producers (DMA from f32r DRAM, or DVE/ACT rounding
    # copies) -- the BIR verifier enforces this.
    mmdt = getattr(mybir.dt, mm_dtype_name)

    nc = bacc.Bacc()
    x1s = nc.declare_dram_parameter("x1s", [NI, HW], f32, isOutput=False)
    x2d = nc.declare_dram_parameter("x2", [NW, NR, HW], mmdt, isOutput=False)
    out = nc.declare_dram_parameter("out", [NI, NW], f32, isOutput=True)

    AF = mybir.ActivationFunctionType
    OP = mybir.AluOpType

    with tile.TileContext(nc) as tc:
        with ExitStack() as ctx:
            persist = ctx.enter_context(tc.tile_pool(name="persist", bufs=1))
            ident = persist.tile([128, 128], f32, tag="ident")
            make_identity(nc, ident)
            ones_f = persist.tile([128, 1], f32, tag="ones_f")
            nc.vector.memset(ones_f, 1.0)
            ones = persist.tile([128, 1], mmdt, tag="ones")
            nc.vector.tensor_copy(out=ones, in_=ones_f)
            # D stays resident: d_res[:, m, q] = D[m*128 + p, q]
            d_res = persist.tile([128, MT, HW], f32, tag="d_res")
            out_acc = persist.tile([128, MT, NW], f32, tag="out_acc")
            if stages != "full":
                nc.vector.memset(out_acc, 0.0)

            dram = ctx.enter_context(tc.tile_pool(name="dram", bufs=1, space="DRAM"))
            # dtT_dram[m][p, kt, i] = D[m*128 + i, kt*128 + p] (full k-blocks)
            dtT_dram = dram.tile([MT, 128, NKT - 1, 128], mmdt, tag="dtT")
            # remainder k-block (16 rows of p)
            dtr_dram = dram.tile([MT, KT[-1][1], 128], mmdt, tag="dtr")

            scr_pool = ctx.enter_context(tc.tile_pool(name="scr", bufs=2))
            stats = ctx.enter_context(tc.tile_pool(name="stats", bufs=4))

            # Optional on-device repeat loop (timing amplification only).
            if repeat > 1:
                ctx.enter_context(tc.For_i(0, repeat, 1))

            # ---- Stage 0: query preprocessing + D^T build ----
            with tc.tile_pool(name="psum_t", bufs=2, space="PSUM") as psum_t, \
                 tc.tile_pool(name="dtw", bufs=2) as dtw_pool:
                for m in range(MT):
                    rows = slice(m * 128, (m + 1) * 128)
                    dsl = d_res[:, m, :]
                    nc.sync.dma_start(out=dsl, in_=x1s[rows, :])
                    sq = scr_pool.tile([128, HW], f32, tag="scr")
                    sumsq = stats.tile([128, 1], f32, tag="sumsq")
                    # ACT: sq = x^2 (discarded), sumsq = row-sum(x^2)
                    nc.scalar.activation(
                        out=sq, in_=dsl, func=AF.Square, accum_out=sumsq
                    )
                    s1 = stats.tile([128, 1], f32, tag="s1")
                    nc.vector.tensor_reduce(
                        out=s1, in_=dsl, axis=mybir.AxisListType.X, op=OP.add
                    )
                    nrm = stats.tile([128, 1], f32, tag="nrm")
                    nc.scalar.activation(out=nrm, in_=sumsq, func=AF.Sqrt)
                    rn = stats.tile([128, 1], f32, tag="rn")
                    nc.vector.reciprocal(out=rn, in_=nrm)
                    ms = stats.tile([128, 1], f32, tag="ms")
                    nc.scalar.mul(out=ms, in_=s1, mul=1.0 / HW)
                    # d = (x - mean) * (1/||x||), in place
                    nc.vector.tensor_scalar(
                        out=dsl, in0=dsl, scalar1=ms, scalar2=rn,
                        op0=OP.subtract, op1=OP.mult,
                    )
                    dtw = dtw_pool.tile([128, NKT - 1, 128], mmdt, tag="dtw")
                    dtr = dtw_pool.tile([KT[-1][1], 128], mmdt, tag="dtr")
                    for kt, (koff, klen) in enumerate(KT):
                        pt = psum_t.tile([128, 128], f32, tag="pt")
                        nc.tensor.transpose(
                            out=pt[:klen, :128],
                            in_=dsl[:, koff:koff + klen],
                            identity=ident,
                        )
                        dst = dtw[:, kt, :] if kt < NKT - 1 else dtr
                        nc.vector.tensor_copy(out=dst, in_=pt[:klen, :128])
                    nc.sync.dma_start(out=dtT_dram[m], in_=dtw)
                    nc.sync.dma_start(out=dtr_dram[m], in_=dtr)

            # ---- Stages 1+2 per class ----
            xs_pool = ctx.enter_context(tc.tile_pool(name="xsup", bufs=1))
            cov_pool = ctx.enter_context(tc.tile_pool(name="cov", bufs=2))
            row_pool = ctx.enter_context(tc.tile_pool(name="rows", bufs=1))
            dts_pool = ctx.enter_context(tc.tile_pool(name="dts", bufs=3))
            psum_s = ctx.enter_context(
                tc.tile_pool(name="psum_s", bufs=4, space="PSUM")
            )
            psum_m = ctx.enter_context(
                tc.tile_pool(name="psum_mean", bufs=1, space="PSUM")
            )

            RTN = NR // 128  # 10 row-tiles per class
            for n in range(NW if stages != "0" else 0):
                xs = xs_pool.tile([128, RTN, HW], mmdt, tag="xs")
                for rt in range(RTN):
                    nc.sync.dma_start(
                        out=xs[:, rt, :], in_=x2d[n, rt * 128:(rt + 1) * 128, :]
                    )
                # column sums s (1, HW) via ones-matmul; psum sub-tiles are
                # bank-aligned (512-elem stride) so no matmul crosses a bank.
                pm = psum_m.tile([1, len(QT), 512], f32, tag="pm")
                for rt in range(RTN):
                    for qi, (qoff, qlen) in enumerate(QT):
                        nc.tensor.matmul(
                            pm[:1, qi, :qlen],
                            lhsT=ones[:, :1],
                            rhs=xs[:, rt, qoff:qoff + qlen],
                            start=(rt == 0),
                            stop=(rt == RTN - 1),
                        )
                srow = row_pool.tile([1, HW], mmdt, tag="srow")
                ssrow = row_pool.tile([1, HW], mmdt, tag="ssrow")
                for qi, (qoff, qlen) in enumerate(QT):
                    qs = slice(qoff, qoff + qlen)
                    nc.scalar.mul(out=srow[:, qs], in_=pm[:1, qi, :qlen], mul=1.0)
                    nc.scalar.mul(out=ssrow[:, qs], in_=pm[:1, qi, :qlen], mul=-1.0 / NR)

                # cov_n = (X^T X - s s^T / NR) / (NR - 1), tiled (p-block, q)
                cov = cov_pool.tile([128, NKT, HW], mmdt, tag="cov")
                for mc, (mcoff, mclen) in enumerate(KT):
                    for (qoff, qlen) in QT:
                        ps = psum_s.tile([128, 392], f32, tag="ps")
                        for rt in range(RTN):
                            nc.tensor.matmul(
                                ps[:mclen, :qlen],
                                lhsT=xs[:, rt, mcoff:mcoff + mclen],
                                rhs=xs[:, rt, qoff:qoff + qlen],
                                start=(rt == 0),
                                stop=False,
                            )
                        nc.tensor.matmul(
                            ps[:mclen, :qlen],
                            lhsT=ssrow[:1, mcoff:mcoff + mclen],
                            rhs=srow[:1, qoff:qoff + qlen],
                            start=False,
                            stop=True,
                        )
                        nc.scalar.mul(
                            out=cov[:mclen, mc, qoff:qoff + qlen],
                            in_=ps[:mclen, :qlen],
                            mul=1.0 / (NR - 1),
                        )

                # Stage 2: sim[:, n] = rowsum((D @ cov_n) * D) per m-tile
                for m in range(MT if stages not in ("0", "01") else 0):
                    dtw2 = dts_pool.tile([128, NKT - 1, 128], mmdt, tag="dts")
                    nc.sync.dma_start(out=dtw2, in_=dtT_dram[m])
                    dtr2 = dts_pool.tile([KT[-1][1], 128], mmdt, tag="dtsr")
                    nc.sync.dma_start(out=dtr2, in_=dtr_dram[m])
                    if stages == "2d":
                        scr = scr_pool.tile([128, HW], f32, tag="scr")
                        nc.vector.tensor_copy(out=scr[:, :128], in_=dtw2[:, 0, :])
                        continue
                    if stages == "2w":
                        # matmuls with weights from a fixed resident tile
                        # (no dependence on the streamed dtw2) - stall probe
                        for qi, (qoff, qlen) in enumerate(QT):
                            ps = psum_s.tile([128, 392], f32, name="ps", tag="ps")
                            for kt, (koff, klen) in enumerate(KT):
                                nc.tensor.matmul(
                                    ps[:128, :qlen],
                                    lhsT=xs[:klen, 0, :128],
                                    rhs=cov[:klen, kt, qoff:qoff + qlen],
                                    start=(kt == 0),
                                    stop=(kt == NKT - 1),
                                )
                            scr = scr_pool.tile([128, HW], f32, tag="scr")
                            nc.vector.tensor_copy(out=scr[:, :qlen], in_=ps[:, :qlen])
                        continue
                    acc = out_acc[:, m, n:n + 1]
                    pp = stats.tile([128, 2], f32, name="pp", tag="pp")
                    for qi, (qoff, qlen) in enumerate(QT):
                        ps = psum_s.tile([128, 392], f32, tag="ps")
                        for kt, (koff, klen) in enumerate(KT):
                            lhsT = dtw2[:, kt, :] if kt < NKT - 1 else dtr2
                            nc.tensor.matmul(
                                ps[:128, :qlen],
                                lhsT=lhsT,
                                rhs=cov[:klen, kt, qoff:qoff + qlen],
                                start=(kt == 0),
                                stop=(kt == NKT - 1),
                            )
                        scr = scr_pool.tile([128, HW], f32, tag="scr")
                        # out=(ps*1)*d elementwise; accum_out = row-sum
                        nc.vector.scalar_tensor_tensor(
                            out=scr[:, :qlen],
                            in0=ps[:, :qlen],
                            scalar=1.0,
                            in1=d_res[:, m, qoff:qoff + qlen],
                            op0=OP.mult,
                            op1=OP.mult,
                            accum_out=pp[:, qi:qi + 1],
                        )
                    nc.vector.tensor_reduce(
                        out=acc, in_=pp, axis=mybir.AxisListType.X, op=OP.add
                    )

            for m in range(MT):
                nc.sync.dma_start(
                    out=out[m * 128:(m + 1) * 128, :], in_=out_acc[:, m, :]
                )

    # Bacc defers register allocation to compile(); run_bass_via_pjrt
    # serializes the module as-is, so finalize here.
    nc.finalize()
    return nc


def get_program():
    key = "nc"
    if key not in _STATE:
        _STATE[key] = _build_program(
            os.environ.get("CCB_MM_DTYPE", "float32r")
        )
    return _STATE[key]


def make_in_maps(x1, x2):
    x1f = np.ascontiguousarray(
        np.asarray(x1, dtype=np.float32).reshape(B * C, HW)
    )
    x2f = np.ascontiguousarray(
        np.asarray(x2, dtype=np.float32).reshape(NW, NR, HW)
    )
    return [
        {"x1s": x1f[c * NI:(c + 1) * NI], "x2": x2f}
        for c in range(N_CORES)
    ]


def assemble_output(core_outs):
    # per-core (NI, NW) -> (BSH, NW*C); concat over cores -> (B, NW*C)
    parts = [
        o.reshape(BSH, C, NW).transpose(0, 2, 1).reshape(BSH, NW * C)
        for o in core_outs
    ]
    return np.ascontiguousarray(np.concatenate(parts, axis=0), dtype=np.float32)


def kernel(x1, x2):
    from concourse.bass_utils import run_bass_kernel_spmd

    nc = get_program()
    in_maps = make_in_maps(x1, x2)
    res = run_bass_kernel_spmd(nc, in_maps, list(range(N_CORES)))
    return assemble_output([res.results[i]["out"] for i in range(N_CORES)])
